# revision 11
# baseline (speedup 1.0000x reference)
"""SINDy autoencoder forward pass on 8 Trainium2 NeuronCores.

Data-parallel: batch (16384) sharded 8 ways, feature-major on device
(matmuls contract over the SBUF partition dim).

Key layout tricks (v2):
- Host interleaves x'' = (x - db4) and x_dot into one tiled tensor
  xxd[b-tile, partition, k-chunk, 0:256|256:512] so each batch tile loads
  with ONE sequential-DRAM DMA, and the forward + forward-derivative
  matmuls share a fused N=512 rhs ([act | deriv] halves of one tile).
  The db4 shift makes the decoder output x_hat0 = W4 h3 (no bias)
  directly comparable: recon = sum((x'' - x_hat0)^2); eb1 absorbs the
  shift (eb1' = eb1 + eW1[:, :D] @ db4).
- The x_hat output is returned as dif = x_hat0 - x''; the host adds x
  back (x_hat = dif + x), which removes all PSUM->SBUF output copies.
- Matmul operands are float32r (TRN2 fast fp32 mode, ~tf32 rounding).
- Loss partials: fused square+row-sum accumulators, alternating between
  the ACT and DVE engines; sigmoid-derivative prep runs on GpSimd.
"""

import numpy as np

import sys

if "/opt/trn_rl_repo" not in sys.path:
    sys.path.insert(0, "/opt/trn_rl_repo")

from contextlib import ExitStack

import concourse.bacc as bacc
import concourse.mybir as mybir
from concourse import tile
from concourse.bass_utils import run_bass_kernel_spmd

F32 = mybir.dt.float32
F32R = mybir.dt.float32r
AF = mybir.ActivationFunctionType
OP = mybir.AluOpType

B = 16384
D = 2048
NC_CORES = 8
BS = B // NC_CORES   # 2048 rows per core
NT = 256             # batch tile
NW = 2 * NT          # fused (act | deriv) width
NBT = BS // NT       # 8 batch tiles
KC = D // 128        # 16 input chunks
MC = D // 128        # 16 output chunks
PBT = 40
# partials cols per b-tile: [0:16) recon, [16:32) sindy_x, 32 po, 33 tr,
# 34 sindy_z (rows 0:3)

SW_W2 = (128, 0, 64)
SW_W3 = (64, 64, 32)
SW_W4 = (32, 96, 3)
SW_V1 = (3, 99, 32)
SW_V2 = (32, 131, 64)
SW_V3 = (64, 195, 128)
SW_DB4 = (128, 323, 16)
SW_EB1 = (128, 339, 1)
SW_EB2 = (64, 340, 1)
SW_EB3 = (32, 341, 1)
SW_EB4 = (3, 342, 1)
SW_DB1 = (32, 343, 1)
SW_DB2 = (64, 344, 1)
SW_DB3 = (128, 345, 1)
SW_SELH = (3, 346, 3)
SW_SELL = (3, 349, 1)
SW_CZ = (3, 350, 3)
SW_CP = (3, 353, 3)
SW_CQ = (3, 356, 3)
SW_C0 = (3, 359, 1)
SW_COLS = 360

_BUILt = None


def build():
    global _BUILt
    if _BUILt is not None:
        return _BUILt

    nc = bacc.Bacc("TRN2", target_bir_lowering=False, debug=False,
                   num_devices=NC_CORES)

    xxd_d = nc.dram_tensor("xxd", [NBT, 128, KC, NW], F32R,
                           kind="ExternalInput")
    trt_d = nc.dram_tensor("trt2", [NBT, NW], F32R, kind="ExternalInput")
    sz_d = nc.dram_tensor("size", [NBT, NT], F32, kind="ExternalInput")
    w1_d = nc.dram_tensor("w1T", [D + 1, 128], F32R, kind="ExternalInput")
    v4_d = nc.dram_tensor("v4T", [128, D], F32R, kind="ExternalInput")
    sw_d = nc.dram_tensor("smallw", [128, SW_COLS], F32R,
                          kind="ExternalInput")

    z_d = nc.dram_tensor("zT", [NBT, 3, NT], F32, kind="ExternalOutput")
    dif_d = nc.dram_tensor("difT", [NBT, 128, KC, NT], F32,
                           kind="ExternalOutput")
    pr_d = nc.dram_tensor("partials", [128, NBT * PBT], F32,
                          kind="ExternalOutput")

    def f(ap):
        return ap.bitcast(F32)

    with tile.TileContext(nc) as tc, ExitStack() as ctx:
        wp = ctx.enter_context(tc.tile_pool(name="w", bufs=1))
        xp = ctx.enter_context(tc.tile_pool(name="x", bufs=2))
        ap_ = ctx.enter_context(tc.tile_pool(name="act", bufs=2))
        gp_ = ctx.enter_context(tc.tile_pool(name="gp", bufs=2))
        dp = ctx.enter_context(tc.tile_pool(name="dall", bufs=1))
        cp_ = ctx.enter_context(tc.tile_pool(name="accs", bufs=2))
        pp = ctx.enter_context(tc.tile_pool(name="ps", bufs=8, space="PSUM"))

        # ---- weights (loaded once) ----------------------------------------
        w1_sb = wp.tile([128, KC * 128], F32R)
        for k in range(KC):
            nc.sync.dma_start(out=w1_sb[:, k * 128:(k + 1) * 128],
                              in_=w1_d[k * 128:(k + 1) * 128, :])
        w1r_sb = wp.tile([1, 128], F32R)
        nc.sync.dma_start(out=w1r_sb[:], in_=w1_d[D:D + 1, :])
        v4_sb = wp.tile([128, D], F32R)
        nc.sync.dma_start(out=v4_sb[:], in_=v4_d[:, :])
        sw = wp.tile([128, SW_COLS], F32R)
        nc.sync.dma_start(out=sw[:], in_=sw_d[:, :])

        def swslice(spec):
            p, c0, w = spec
            return sw[0:p, c0:c0 + w]

        w2 = swslice(SW_W2)
        w3 = swslice(SW_W3)
        w4 = swslice(SW_W4)
        v1 = swslice(SW_V1)
        v2 = swslice(SW_V2)
        v3 = swslice(SW_V3)
        eb1 = f(swslice(SW_EB1))
        eb2 = f(swslice(SW_EB2))
        eb3 = f(swslice(SW_EB3))
        eb4 = f(swslice(SW_EB4))
        db1 = f(swslice(SW_DB1))
        db2 = f(swslice(SW_DB2))
        db3 = f(swslice(SW_DB3))
        selh = swslice(SW_SELH)
        sell = swslice(SW_SELL)
        cz = swslice(SW_CZ)
        cpm = swslice(SW_CP)
        cq = swslice(SW_CQ)
        c0 = f(swslice(SW_C0))

        def layer_pair(name, ps, lo, hi, bias, width):
            """Fused tile [act | deriv] from a fused PSUM [pre | dpre]."""
            t = ap_.tile([lo, NW], F32R, tag=name, name=f"{name}_sb")
            nc.scalar.activation(t[:, 0:NT], ps[:, 0:NT], AF.Sigmoid,
                                 bias=bias)
            sq = gp_.tile([lo, NT], F32, tag=f"sq{width}", name=f"{name}_sq")
            nc.gpsimd.tensor_mul(sq[:], f(t[:, 0:NT]), f(t[:, 0:NT]))
            g = gp_.tile([lo, NT], F32, tag=f"g{width}", name=f"{name}_g")
            nc.gpsimd.tensor_sub(g[:], f(t[:, 0:NT]), sq[:])
            nc.vector.tensor_mul(t[:, NT:NW], g[:], ps[:, NT:NW])
            return t

        for bt in range(NBT):
            # ---- loads (one sequential DMA for the fused x tensor) --------
            xxd = xp.tile([128, KC * NW], F32R, tag="xxd")
            nc.sync.dma_start(
                out=xxd[:].rearrange("p (n b) -> p n b", n=KC),
                in_=xxd_d[bt])
            trt = xp.tile([1, NW], F32R, tag="trt")
            nc.sync.dma_start(out=trt[:], in_=trt_d[bt:bt + 1, :])
            sz = xp.tile([1, NT], F32, tag="sz")
            nc.sync.dma_start(out=sz[:], in_=sz_d[bt:bt + 1, :])

            def xck(k):
                return xxd[:, k * NW:(k + 1) * NW]

            # ---- encoder (fused fwd | deriv) ------------------------------
            ps1 = pp.tile([128, NW], F32, tag="ps")
            for k in range(KC):
                nc.tensor.matmul(ps1[:], w1_sb[:, k * 128:(k + 1) * 128],
                                 xck(k), start=(k == 0), stop=False)
            nc.tensor.matmul(ps1[:], w1r_sb[:], trt[:], start=False,
                             stop=True)
            adz1 = layer_pair("adz1", ps1, 128, None, eb1, 1)

            ps2 = pp.tile([64, NW], F32, tag="ps")
            nc.tensor.matmul(ps2[:], w2, adz1[:], start=True, stop=True)
            adz2 = layer_pair("adz2", ps2, 64, None, eb2, 2)

            ps3 = pp.tile([32, NW], F32, tag="ps")
            nc.tensor.matmul(ps3[:], w3, adz2[:], start=True, stop=True)
            adz3 = layer_pair("adz3", ps3, 32, None, eb3, 3)

            ps4 = pp.tile([3, NW], F32, tag="ps")   # [z | z_dot_true]
            nc.tensor.matmul(ps4[:], w4, adz3[:], start=True, stop=True)

            zz = ap_.tile([3, NW], F32R, tag="zz")  # [z | z_dot_pred]
            nc.scalar.activation(zz[:, 0:NT], ps4[:, 0:NT], AF.Identity,
                                 bias=eb4)
            nc.sync.dma_start(out=z_d[bt], in_=f(zz[0:3, 0:NT]))

            # ---- SINDy z_dot_pred -----------------------------------------
            hrep_ps = pp.tile([3, NT], F32, tag="ps")
            nc.tensor.matmul(hrep_ps[:], selh, zz[:, 0:NT], start=True,
                             stop=True)
            p3 = ap_.tile([3, NT], F32R, tag="p3")
            nc.vector.tensor_mul(p3[:], f(zz[0:3, 0:NT]), hrep_ps[:])
            p3b = ap_.tile([3, NT], F32R, tag="p3b")
            nc.vector.tensor_mul(p3b[:], f(p3[:]), hrep_ps[:])
            zdp_ps = pp.tile([3, NT], F32, tag="ps")
            nc.tensor.matmul(zdp_ps[:], cz, zz[:, 0:NT], start=True,
                             stop=False)
            nc.tensor.matmul(zdp_ps[:], cpm, p3[:], start=False, stop=False)
            nc.tensor.matmul(zdp_ps[:], cq, p3b[:], start=False, stop=True)
            nc.scalar.activation(zz[:, NT:NW], zdp_ps[:], AF.Identity,
                                 bias=c0)

            # ---- loss partials (tiny) -------------------------------------
            acc = cp_.tile([128, PBT], F32, tag="acc")
            nc.vector.memset(acc[:, 32:35], 0.0)

            dsz = ap_.tile([3, NT], F32, tag="dsz")
            nc.vector.tensor_sub(dsz[:], ps4[:, NT:NW], f(zz[0:3, NT:NW]))
            dszs = ap_.tile([3, NT], F32, tag="dszs")
            nc.scalar.activation(dszs[:], dsz[:], AF.Square,
                                 accum_out=acc[0:3, 34:35])

            dpo = ap_.tile([1, NT], F32, tag="dpo")
            nc.vector.tensor_sub(dpo[:], f(zz[0:1, 0:NT]), sz[:])
            dpos = ap_.tile([1, NT], F32, tag="dpos")
            nc.scalar.activation(dpos[:], dpo[:], AF.Square,
                                 accum_out=acc[0:1, 32:33])

            # loss_tr = sum(-ln(sigmoid(-l)) - l*t)  (host negates)
            lrep_ps = pp.tile([1, NT], F32, tag="ps")
            nc.tensor.matmul(lrep_ps[:], sell, zz[:, 0:NT], start=True,
                             stop=True)
            sneg = ap_.tile([1, NT], F32, tag="sneg")
            nc.scalar.activation(sneg[:], lrep_ps[:], AF.Sigmoid, scale=-1.0)
            lnn = ap_.tile([1, NT], F32, tag="lnn")
            nc.scalar.activation(lnn[:], sneg[:], AF.Ln)
            lt = ap_.tile([1, NT], F32, tag="lt")
            nc.vector.tensor_mul(lt[:], lrep_ps[:], f(trt[0:1, 0:NT]))
            trw = ap_.tile([1, NT], F32, tag="trw")
            nc.vector.scalar_tensor_tensor(trw[:], lnn[:], 1.0, lt[:],
                                           OP.mult, OP.add,
                                           accum_out=acc[0:1, 33:34])

            # ---- decoder (fused fwd | deriv) ------------------------------
            pd1 = pp.tile([32, NW], F32, tag="ps")
            nc.tensor.matmul(pd1[:], v1, zz[:], start=True, stop=True)
            hd1 = layer_pair("hd1", pd1, 32, None, db1, 4)

            pd2 = pp.tile([64, NW], F32, tag="ps")
            nc.tensor.matmul(pd2[:], v2, hd1[:], start=True, stop=True)
            hd2 = layer_pair("hd2", pd2, 64, None, db2, 5)

            pd3 = pp.tile([128, NW], F32, tag="ps")
            nc.tensor.matmul(pd3[:], v3, hd2[:], start=True, stop=True)
            hd3 = layer_pair("hd3", pd3, 128, None, db3, 6)

            # ---- decoder layer 4 chunks -----------------------------------
            dall = dp.tile([128, KC * NW], F32, tag="dall")
            for m in range(MC):
                psm = pp.tile([128, NW], F32, tag="ps")
                nc.tensor.matmul(psm[:], v4_sb[:, m * 128:(m + 1) * 128],
                                 hd3[:], start=True, stop=True)
                dm = dall[:, m * NW:(m + 1) * NW]
                nc.vector.tensor_sub(dm, psm[:], f(xck(m)))
                # alternate the two square+accum passes across ACT/DVE
                ra = acc[:, m:m + 1]
                sa = acc[:, 16 + m:17 + m]
                if m % 2 == 0:
                    scr = ap_.tile([128, NT], F32, tag="scr",
                                   name=f"scr_{bt}_{m}")
                    nc.scalar.activation(scr[:], dm[:, 0:NT], AF.Square,
                                         accum_out=ra)
                    scr2 = ap_.tile([128, NT], F32, tag="scr2",
                                    name=f"scr2_{bt}_{m}")
                    nc.vector.scalar_tensor_tensor(scr2[:], dm[:, NT:NW],
                                                   1.0, dm[:, NT:NW],
                                                   OP.mult, OP.mult,
                                                   accum_out=sa)
                else:
                    scr = ap_.tile([128, NT], F32, tag="scr",
                                   name=f"scr_{bt}_{m}")
                    nc.vector.scalar_tensor_tensor(scr[:], dm[:, 0:NT],
                                                   1.0, dm[:, 0:NT],
                                                   OP.mult, OP.mult,
                                                   accum_out=ra)
                    scr2 = ap_.tile([128, NT], F32, tag="scr2",
                                    name=f"scr2_{bt}_{m}")
                    nc.scalar.activation(scr2[:], dm[:, NT:NW], AF.Square,
                                         accum_out=sa)

            # ---- stores ---------------------------------------------------
            nc.sync.dma_start(
                out=dif_d[bt],
                in_=dall[:].rearrange("p (n b) -> p n b", n=KC)[:, :, 0:NT])
            pc = bt * PBT
            nc.sync.dma_start(out=pr_d[:, pc:pc + 35], in_=acc[:, 0:35])

    nc.compile()
    _BUILt = nc
    return nc


def prep_inputs(x, x_dot, treatment, size,
                eW1, eb1, eW2, eb2, eW3, eb3, eW4, eb4,
                dW1, db1, dW2, db2, dW3, db3, dW4, db4, coefficients):
    f32 = np.float32
    x = np.asarray(x, f32)
    x_dot = np.asarray(x_dot, f32)
    db4 = np.asarray(db4, f32)

    # x'' = x - db4 (db4 folded into eb1 and removed from the recon path)
    xsT = np.ascontiguousarray((x - db4[None, :]).T)          # [D, B]
    xdT = np.ascontiguousarray(x_dot.T)                       # [D, B]

    eb1_adj = np.asarray(eb1, f32) + np.asarray(eW1, f32)[:, :D] @ db4

    w1T = np.ascontiguousarray(eW1.T, f32)
    v4T = np.ascontiguousarray(dW4.T, f32)

    sw = np.zeros((128, SW_COLS), f32)

    def put(spec, val):
        p, c0, w = spec
        assert val.shape == (p, w), (spec, val.shape)
        sw[0:p, c0:c0 + w] = val

    C = np.asarray(coefficients, f32)
    selh = np.zeros((3, 3), f32)
    selh[0, :] = 1.0
    sell = np.zeros((3, 1), f32)
    sell[2, 0] = 1.0
    cz = np.zeros((3, 3), f32)
    cz[0, :] = C[1, :]
    cq = np.zeros((3, 3), f32)
    cq[1, :] = C[5, :]
    cq[2, :] = C[6, :]

    put(SW_W2, eW2.T)
    put(SW_W3, eW3.T)
    put(SW_W4, eW4.T)
    put(SW_V1, dW1.T)
    put(SW_V2, dW2.T)
    put(SW_V3, dW3.T)
    put(SW_DB4, np.ascontiguousarray(db4.reshape(MC, 128).T))
    put(SW_EB1, eb1_adj.reshape(-1, 1))
    put(SW_EB2, eb2.reshape(-1, 1))
    put(SW_EB3, eb3.reshape(-1, 1))
    put(SW_EB4, eb4.reshape(-1, 1))
    put(SW_DB1, db1.reshape(-1, 1))
    put(SW_DB2, db2.reshape(-1, 1))
    put(SW_DB3, db3.reshape(-1, 1))
    put(SW_SELH, selh)
    put(SW_SELL, sell)
    put(SW_CZ, cz)
    put(SW_CP, np.ascontiguousarray(C[2:5, :]))
    put(SW_CQ, cq)
    put(SW_C0, C[0, :].reshape(3, 1))

    treat = np.asarray(treatment, f32)[:, 0]
    sizeT = np.asarray(size, f32)[:, 0]

    in_maps = []
    for c in range(NC_CORES):
        sl = slice(c * BS, (c + 1) * BS)
        # [D, BS] -> [KC, 128, NBT, NT] -> [NBT, 128, KC, NT]
        ax = xsT[:, sl].reshape(KC, 128, NBT, NT).transpose(2, 1, 0, 3)
        ad = xdT[:, sl].reshape(KC, 128, NBT, NT).transpose(2, 1, 0, 3)
        xxd = np.empty((NBT, 128, KC, NW), f32)
        xxd[..., 0:NT] = ax
        xxd[..., NT:NW] = ad
        t2 = np.empty((NBT, NW), f32)
        t2[:, 0:NT] = treat[sl].reshape(NBT, NT)
        t2[:, NT:NW] = t2[:, 0:NT]
        in_maps.append({
            "xxd": xxd,
            "trt2": t2,
            "size": np.ascontiguousarray(sizeT[sl].reshape(NBT, NT)),
            "w1T": w1T,
            "v4T": v4T,
            "smallw": sw,
        })
    return in_maps


def finish(results, x, coefficients):
    f32 = np.float32
    x = np.asarray(x, f32)
    z = np.empty((B, 3), f32)
    x_hat = np.empty((B, D), f32)
    po = tr = rec = sx = szz = 0.0
    for c in range(NC_CORES):
        sl = slice(c * BS, (c + 1) * BS)
        r = results[c]
        z[sl] = r["zT"].transpose(0, 2, 1).reshape(BS, 3)
        dif = r["difT"]        # [NBT, 128, KC, NT]
        x_hat[sl] = dif.transpose(0, 3, 2, 1).reshape(BS, D) + x[sl]
        P = r["partials"].reshape(128, NBT, PBT).astype(np.float64)
        rec += P[:, :, 0:16].sum()
        sx += P[:, :, 16:32].sum()
        po += P[0, :, 32].sum()
        tr -= P[0, :, 33].sum()     # device computed -(softplus - l*t)
        szz += P[0:3, :, 34].sum()
    loss_po = np.float32(po / B)
    loss_tr = np.float32(tr / B)
    recon = np.float32(rec / (B * D))
    sindy_x = np.float32(sx / (B * D))
    sindy_z = np.float32(szz / (B * 3))
    l1 = np.float32(np.mean(np.abs(np.asarray(coefficients, np.float64))))
    return (z, x_hat, loss_po, loss_tr, recon, sindy_x, sindy_z, l1)


def kernel(**inputs):
    nc = build()
    in_maps = prep_inputs(**inputs)
    res = run_bass_kernel_spmd(nc, in_maps, list(range(NC_CORES)))
    return finish(res.results, inputs["x"], inputs["coefficients"])


# revision 12
# speedup vs baseline: 1.2049x; 1.2049x over previous
"""SINDy autoencoder forward pass on 8 Trainium2 NeuronCores.

Data-parallel: batch (16384) sharded 8 ways, feature-major on device
(matmuls contract over the SBUF partition dim).

v3 structure:
- Host interleaves x'' = (x - db4) and x_dot into one tiled tensor
  xxd[b-tile, partition, k-chunk, 0:256|256:512]: one sequential-DRAM DMA
  per batch tile, and fused N=512 matmuls whose rhs is [act | deriv].
  eb1 absorbs the db4 shift; recon compares x_hat0 = W4 h3 against x''
  directly, and the x_hat output is dif = x_hat0 - x'' (host adds x).
- Software-pipelined emission: loads run 3 tiles ahead, the next tile's
  wide layer-1 matmul group is emitted before the current tile's serial
  layer chain, so the PE fills the chain's dependency stalls.
- float32r matmul operands (TRN2 fast fp32 mode); losses in fp32 via
  fused square+row-sum accumulators split across ACT/DVE.
"""

import numpy as np

import sys

if "/opt/trn_rl_repo" not in sys.path:
    sys.path.insert(0, "/opt/trn_rl_repo")

from contextlib import ExitStack

import concourse.bacc as bacc
import concourse.mybir as mybir
from concourse import tile
from concourse.bass_utils import run_bass_kernel_spmd

F32 = mybir.dt.float32
F32R = mybir.dt.float32r
AF = mybir.ActivationFunctionType
OP = mybir.AluOpType

B = 16384
D = 2048
NC_CORES = 8
BS = B // NC_CORES   # 2048 rows per core
NT = 256             # batch tile
NW = 2 * NT          # fused (act | deriv) width
NBT = BS // NT       # 8 batch tiles
KC = D // 128        # 16 input chunks
MC = D // 128        # 16 output chunks
PBT = 40
# partials cols per b-tile: [0:16) recon, [16:32) sindy_x, 32 po, 33 tr,
# 34 sindy_z (rows 0:3)

SW_W2 = (128, 0, 64)
SW_W3 = (64, 64, 32)
SW_W4 = (32, 96, 3)
SW_V1 = (3, 99, 32)
SW_V2 = (32, 131, 64)
SW_V3 = (64, 195, 128)
SW_DB4 = (128, 323, 16)
SW_EB1 = (128, 339, 1)
SW_EB2 = (64, 340, 1)
SW_EB3 = (32, 341, 1)
SW_EB4 = (3, 342, 1)
SW_DB1 = (32, 343, 1)
SW_DB2 = (64, 344, 1)
SW_DB3 = (128, 345, 1)
SW_SELH = (3, 346, 3)
SW_SELL = (3, 349, 1)
SW_CZ = (3, 350, 3)
SW_CP = (3, 353, 3)
SW_CQ = (3, 356, 3)
SW_C0 = (3, 359, 1)
SW_COLS = 360

_BUILt = None


def build():
    global _BUILt
    if _BUILt is not None:
        return _BUILt

    nc = bacc.Bacc("TRN2", target_bir_lowering=False, debug=False,
                   num_devices=NC_CORES)

    xxd_d = nc.dram_tensor("xxd", [NBT, 128, KC, NW], F32R,
                           kind="ExternalInput")
    trt_d = nc.dram_tensor("trt2", [NBT, NW], F32R, kind="ExternalInput")
    sz_d = nc.dram_tensor("size", [NBT, NT], F32, kind="ExternalInput")
    w1_d = nc.dram_tensor("w1T", [D + 1, 128], F32R, kind="ExternalInput")
    v4_d = nc.dram_tensor("v4T", [128, D], F32R, kind="ExternalInput")
    sw_d = nc.dram_tensor("smallw", [128, SW_COLS], F32R,
                          kind="ExternalInput")

    z_d = nc.dram_tensor("zT", [NBT, 3, NT], F32, kind="ExternalOutput")
    dif_d = nc.dram_tensor("difT", [NBT, KC, 128, NT], F32,
                           kind="ExternalOutput")
    pr_d = nc.dram_tensor("partials", [128, NBT * PBT], F32,
                          kind="ExternalOutput")

    def f(ap):
        return ap.bitcast(F32)

    with tile.TileContext(nc) as tc, ExitStack() as ctx:
        wp = ctx.enter_context(tc.tile_pool(name="w", bufs=1))
        xp = ctx.enter_context(tc.tile_pool(name="x", bufs=3))
        ap_ = ctx.enter_context(tc.tile_pool(name="act", bufs=2))
        sp = ctx.enter_context(tc.tile_pool(name="strm", bufs=4))
        cp_ = ctx.enter_context(tc.tile_pool(name="accs", bufs=2))
        pp = ctx.enter_context(tc.tile_pool(name="ps", bufs=8, space="PSUM"))

        # ---- weights (loaded once) ----------------------------------------
        w1_sb = wp.tile([128, KC * 128], F32R)
        for k in range(KC):
            nc.sync.dma_start(out=w1_sb[:, k * 128:(k + 1) * 128],
                              in_=w1_d[k * 128:(k + 1) * 128, :])
        w1r_sb = wp.tile([1, 128], F32R)
        nc.sync.dma_start(out=w1r_sb[:], in_=w1_d[D:D + 1, :])
        v4_sb = wp.tile([128, D], F32R)
        nc.sync.dma_start(out=v4_sb[:], in_=v4_d[:, :])
        sw = wp.tile([128, SW_COLS], F32R)
        nc.sync.dma_start(out=sw[:], in_=sw_d[:, :])

        def swslice(spec):
            p, c0, w = spec
            return sw[0:p, c0:c0 + w]

        w2 = swslice(SW_W2)
        w3 = swslice(SW_W3)
        w4 = swslice(SW_W4)
        v1 = swslice(SW_V1)
        v2 = swslice(SW_V2)
        v3 = swslice(SW_V3)
        eb1 = f(swslice(SW_EB1))
        eb2 = f(swslice(SW_EB2))
        eb3 = f(swslice(SW_EB3))
        eb4 = f(swslice(SW_EB4))
        db1 = f(swslice(SW_DB1))
        db2 = f(swslice(SW_DB2))
        db3 = f(swslice(SW_DB3))
        selh = swslice(SW_SELH)
        sell = swslice(SW_SELL)
        cz = swslice(SW_CZ)
        cpm = swslice(SW_CP)
        cq = swslice(SW_CQ)
        c0 = f(swslice(SW_C0))

        def emit_loads(bt):
            xxd = xp.tile([128, KC * NW], F32R, tag="xxd", name=f"xxd{bt}")
            nc.sync.dma_start(
                out=xxd[:].rearrange("p (n b) -> p n b", n=KC),
                in_=xxd_d[bt])
            trt = xp.tile([1, NW], F32R, tag="trt", name=f"trt{bt}")
            nc.sync.dma_start(out=trt[:], in_=trt_d[bt:bt + 1, :])
            sz = xp.tile([1, NT], F32, tag="sz", name=f"sz{bt}")
            nc.sync.dma_start(out=sz[:], in_=sz_d[bt:bt + 1, :])
            return xxd, trt, sz

        def emit_l1(bt, xxd, trt):
            ps1 = pp.tile([128, NW], F32, tag="ps", name=f"ps1_{bt}")
            for k in range(KC):
                nc.tensor.matmul(ps1[:], w1_sb[:, k * 128:(k + 1) * 128],
                                 xxd[:, k * NW:(k + 1) * NW],
                                 start=(k == 0), stop=False)
            nc.tensor.matmul(ps1[:], w1r_sb[:], trt[:], start=False,
                             stop=True)
            return ps1

        def layer_pair(bt, name, ps, lo, bias):
            """Fused [act | deriv] tile from fused PSUM [pre | dpre]."""
            t = ap_.tile([lo, NW], F32R, tag=name, name=f"{name}_{bt}")
            nc.scalar.activation(t[:, 0:NT], ps[:, 0:NT], AF.Sigmoid,
                                 bias=bias)
            g = ap_.tile([lo, NT], F32, tag=f"g_{name}", name=f"g{name}{bt}")
            nc.vector.scalar_tensor_tensor(g[:], f(t[:, 0:NT]), 1.0,
                                           f(t[:, 0:NT]), OP.subtract,
                                           OP.mult)
            nc.vector.scalar_tensor_tensor(t[:, NT:NW], g[:], -1.0,
                                           ps[:, NT:NW], OP.mult, OP.mult)
            return t

        def emit_chain(bt, ps1, trt, sz):
            adz1 = layer_pair(bt, "adz1", ps1, 128, eb1)
            ps2 = pp.tile([64, NW], F32, tag="ps", name=f"ps2_{bt}")
            nc.tensor.matmul(ps2[:], w2, adz1[:], start=True, stop=True)
            adz2 = layer_pair(bt, "adz2", ps2, 64, eb2)
            ps3 = pp.tile([32, NW], F32, tag="ps", name=f"ps3_{bt}")
            nc.tensor.matmul(ps3[:], w3, adz2[:], start=True, stop=True)
            adz3 = layer_pair(bt, "adz3", ps3, 32, eb3)
            ps4 = pp.tile([3, NW], F32, tag="ps", name=f"ps4_{bt}")
            nc.tensor.matmul(ps4[:], w4, adz3[:], start=True, stop=True)

            zz = ap_.tile([3, NW], F32R, tag="zz", name=f"zz{bt}")
            nc.scalar.activation(zz[:, 0:NT], ps4[:, 0:NT], AF.Identity,
                                 bias=eb4)
            nc.sync.dma_start(out=z_d[bt], in_=f(zz[0:3, 0:NT]))

            hrep_ps = pp.tile([3, NT], F32, tag="ps", name=f"hr_{bt}")
            nc.tensor.matmul(hrep_ps[:], selh, zz[:, 0:NT], start=True,
                             stop=True)
            p3 = ap_.tile([3, NT], F32R, tag="p3", name=f"p3_{bt}")
            nc.vector.tensor_mul(p3[:], f(zz[0:3, 0:NT]), hrep_ps[:])
            p3b = ap_.tile([3, NT], F32R, tag="p3b", name=f"p3b_{bt}")
            nc.vector.tensor_mul(p3b[:], f(p3[:]), hrep_ps[:])
            zdp_ps = pp.tile([3, NT], F32, tag="ps", name=f"zp_{bt}")
            nc.tensor.matmul(zdp_ps[:], cz, zz[:, 0:NT], start=True,
                             stop=False)
            nc.tensor.matmul(zdp_ps[:], cpm, p3[:], start=False, stop=False)
            nc.tensor.matmul(zdp_ps[:], cq, p3b[:], start=False, stop=True)
            nc.scalar.activation(zz[:, NT:NW], zdp_ps[:], AF.Identity,
                                 bias=c0)

            acc = cp_.tile([128, PBT], F32, tag="acc", name=f"acc{bt}")
            nc.vector.memset(acc[:, 32:35], 0.0)

            dsz = ap_.tile([3, NT], F32, tag="dsz", name=f"dsz{bt}")
            nc.vector.tensor_sub(dsz[:], ps4[:, NT:NW], f(zz[0:3, NT:NW]))
            dszs = ap_.tile([3, NT], F32, tag="dszs", name=f"dszs{bt}")
            nc.scalar.activation(dszs[:], dsz[:], AF.Square,
                                 accum_out=acc[0:3, 34:35])

            dpo = ap_.tile([1, NT], F32, tag="dpo", name=f"dpo{bt}")
            nc.vector.tensor_sub(dpo[:], f(zz[0:1, 0:NT]), sz[:])
            dpos = ap_.tile([1, NT], F32, tag="dpos", name=f"dpos{bt}")
            nc.scalar.activation(dpos[:], dpo[:], AF.Square,
                                 accum_out=acc[0:1, 32:33])

            lrep_ps = pp.tile([1, NT], F32, tag="ps", name=f"lr_{bt}")
            nc.tensor.matmul(lrep_ps[:], sell, zz[:, 0:NT], start=True,
                             stop=True)
            sneg = ap_.tile([1, NT], F32, tag="sneg", name=f"sneg{bt}")
            nc.scalar.activation(sneg[:], lrep_ps[:], AF.Sigmoid, scale=-1.0)
            lnn = ap_.tile([1, NT], F32, tag="lnn", name=f"lnn{bt}")
            nc.scalar.activation(lnn[:], sneg[:], AF.Ln)
            lt = ap_.tile([1, NT], F32, tag="lt", name=f"lt{bt}")
            nc.vector.tensor_mul(lt[:], lrep_ps[:], f(trt[0:1, 0:NT]))
            trw = ap_.tile([1, NT], F32, tag="trw", name=f"trw{bt}")
            nc.vector.scalar_tensor_tensor(trw[:], lnn[:], 1.0, lt[:],
                                           OP.mult, OP.add,
                                           accum_out=acc[0:1, 33:34])

            pd1 = pp.tile([32, NW], F32, tag="ps", name=f"pd1_{bt}")
            nc.tensor.matmul(pd1[:], v1, zz[:], start=True, stop=True)
            hd1 = layer_pair(bt, "hd1", pd1, 32, db1)
            pd2 = pp.tile([64, NW], F32, tag="ps", name=f"pd2_{bt}")
            nc.tensor.matmul(pd2[:], v2, hd1[:], start=True, stop=True)
            hd2 = layer_pair(bt, "hd2", pd2, 64, db2)
            pd3 = pp.tile([128, NW], F32, tag="ps", name=f"pd3_{bt}")
            nc.tensor.matmul(pd3[:], v3, hd2[:], start=True, stop=True)
            hd3 = layer_pair(bt, "hd3", pd3, 128, db3)
            return hd3, acc

        def emit_chunks(bt, xxd, hd3, acc):
            for m in range(MC):
                psm = pp.tile([128, NW], F32, tag="ps", name=f"pm{bt}_{m}")
                nc.tensor.matmul(psm[:], v4_sb[:, m * 128:(m + 1) * 128],
                                 hd3[:], start=True, stop=True)
                dsub = sp.tile([128, NW], F32, tag="dsub",
                               name=f"ds{bt}_{m}")
                nc.vector.tensor_sub(dsub[:], psm[:],
                                     f(xxd[:, m * NW:(m + 1) * NW]))
                nc.sync.dma_start(out=dif_d[bt, m], in_=dsub[:, 0:NT])
                ra = acc[:, m:m + 1]
                sa = acc[:, 16 + m:17 + m]
                scr = sp.tile([128, NT], F32, tag="scr", name=f"sc{bt}_{m}")
                nc.scalar.activation(scr[:], dsub[:, 0:NT], AF.Square,
                                     accum_out=ra)
                scr2 = sp.tile([128, NT], F32, tag="scr2",
                               name=f"sc2{bt}_{m}")
                if m % 2 == 0:
                    nc.vector.scalar_tensor_tensor(scr2[:], dsub[:, NT:NW],
                                                   1.0, dsub[:, NT:NW],
                                                   OP.mult, OP.mult,
                                                   accum_out=sa)
                else:
                    nc.scalar.activation(scr2[:], dsub[:, NT:NW], AF.Square,
                                         accum_out=sa)
            pc = bt * PBT
            nc.sync.dma_start(out=pr_d[:, pc:pc + 35], in_=acc[:, 0:35])

        # ---- software-pipelined emission ----------------------------------
        loads = {}
        loads[0] = emit_loads(0)
        loads[1] = emit_loads(1)
        ps1s = {0: emit_l1(0, loads[0][0], loads[0][1])}
        for bt in range(NBT):
            if bt + 2 < NBT:
                loads[bt + 2] = emit_loads(bt + 2)
            if bt + 1 < NBT:
                ps1s[bt + 1] = emit_l1(bt + 1, loads[bt + 1][0],
                                       loads[bt + 1][1])
            xxd, trt, sz = loads[bt]
            hd3, acc = emit_chain(bt, ps1s.pop(bt), trt, sz)
            emit_chunks(bt, xxd, hd3, acc)

    nc.compile()
    _BUILt = nc
    return nc


def prep_inputs(x, x_dot, treatment, size,
                eW1, eb1, eW2, eb2, eW3, eb3, eW4, eb4,
                dW1, db1, dW2, db2, dW3, db3, dW4, db4, coefficients):
    f32 = np.float32
    x = np.asarray(x, f32)
    x_dot = np.asarray(x_dot, f32)
    db4 = np.asarray(db4, f32)

    xsT = np.ascontiguousarray((x - db4[None, :]).T)          # [D, B]
    xdT = np.ascontiguousarray(x_dot.T)                       # [D, B]

    eb1_adj = np.asarray(eb1, f32) + np.asarray(eW1, f32)[:, :D] @ db4

    w1T = np.ascontiguousarray(eW1.T, f32)
    v4T = np.ascontiguousarray(dW4.T, f32)

    sw = np.zeros((128, SW_COLS), f32)

    def put(spec, val):
        p, c0, w = spec
        assert val.shape == (p, w), (spec, val.shape)
        sw[0:p, c0:c0 + w] = val

    C = np.asarray(coefficients, f32)
    selh = np.zeros((3, 3), f32)
    selh[0, :] = 1.0
    sell = np.zeros((3, 1), f32)
    sell[2, 0] = 1.0
    cz = np.zeros((3, 3), f32)
    cz[0, :] = C[1, :]
    cq = np.zeros((3, 3), f32)
    cq[1, :] = C[5, :]
    cq[2, :] = C[6, :]

    put(SW_W2, eW2.T)
    put(SW_W3, eW3.T)
    put(SW_W4, eW4.T)
    put(SW_V1, dW1.T)
    put(SW_V2, dW2.T)
    put(SW_V3, dW3.T)
    put(SW_DB4, np.ascontiguousarray(db4.reshape(MC, 128).T))
    put(SW_EB1, eb1_adj.reshape(-1, 1))
    put(SW_EB2, eb2.reshape(-1, 1))
    put(SW_EB3, eb3.reshape(-1, 1))
    put(SW_EB4, eb4.reshape(-1, 1))
    put(SW_DB1, db1.reshape(-1, 1))
    put(SW_DB2, db2.reshape(-1, 1))
    put(SW_DB3, db3.reshape(-1, 1))
    put(SW_SELH, selh)
    put(SW_SELL, sell)
    put(SW_CZ, cz)
    put(SW_CP, np.ascontiguousarray(C[2:5, :]))
    put(SW_CQ, cq)
    put(SW_C0, C[0, :].reshape(3, 1))

    treat = np.asarray(treatment, f32)[:, 0]
    sizeT = np.asarray(size, f32)[:, 0]

    in_maps = []
    for c in range(NC_CORES):
        sl = slice(c * BS, (c + 1) * BS)
        ax = xsT[:, sl].reshape(KC, 128, NBT, NT).transpose(2, 1, 0, 3)
        ad = xdT[:, sl].reshape(KC, 128, NBT, NT).transpose(2, 1, 0, 3)
        xxd = np.empty((NBT, 128, KC, NW), f32)
        xxd[..., 0:NT] = ax
        xxd[..., NT:NW] = ad
        t2 = np.empty((NBT, NW), f32)
        t2[:, 0:NT] = treat[sl].reshape(NBT, NT)
        t2[:, NT:NW] = t2[:, 0:NT]
        in_maps.append({
            "xxd": xxd,
            "trt2": t2,
            "size": np.ascontiguousarray(sizeT[sl].reshape(NBT, NT)),
            "w1T": w1T,
            "v4T": v4T,
            "smallw": sw,
        })
    return in_maps


def finish(results, x, coefficients):
    f32 = np.float32
    x = np.asarray(x, f32)
    z = np.empty((B, 3), f32)
    x_hat = np.empty((B, D), f32)
    po = tr = rec = sx = szz = 0.0
    for c in range(NC_CORES):
        sl = slice(c * BS, (c + 1) * BS)
        r = results[c]
        z[sl] = r["zT"].transpose(0, 2, 1).reshape(BS, 3)
        dif = r["difT"]        # [NBT, KC, 128, NT]
        x_hat[sl] = dif.transpose(0, 3, 1, 2).reshape(BS, D) + x[sl]
        P = r["partials"].reshape(128, NBT, PBT).astype(np.float64)
        rec += P[:, :, 0:16].sum()
        sx += P[:, :, 16:32].sum()
        po += P[0, :, 32].sum()
        tr -= P[0, :, 33].sum()
        szz += P[0:3, :, 34].sum()
    loss_po = np.float32(po / B)
    loss_tr = np.float32(tr / B)
    recon = np.float32(rec / (B * D))
    sindy_x = np.float32(sx / (B * D))
    sindy_z = np.float32(szz / (B * 3))
    l1 = np.float32(np.mean(np.abs(np.asarray(coefficients, np.float64))))
    return (z, x_hat, loss_po, loss_tr, recon, sindy_x, sindy_z, l1)


def kernel(**inputs):
    nc = build()
    in_maps = prep_inputs(**inputs)
    res = run_bass_kernel_spmd(nc, in_maps, list(range(NC_CORES)))
    return finish(res.results, inputs["x"], inputs["coefficients"])
